# revision 1
# baseline (speedup 1.0000x reference)
"""Correspondence Soft-NMS on 8 Trainium2 NeuronCores (Bass/Tile).

Math: penalty_i = sum_j [s_j > s_i] * exp(-(d2src_ij + d2tgt_ij)/delta^2)
      out_i    = s_i * exp(-penalty_i / sigma)

Strategy:
  * Host sorts points by score descending. Then {j: s_j > s_i} == {j: j < i}
    (strict positional prefix), up to exact-tie pairs which are fixed by a
    tiny host-side multiplicative correction.
  * d2src+d2tgt folds into one K=7 dot product over P=[src,tgt] (6 dims) plus
    norm terms. Each fp32 operand is split into 3 bf16 limbs -> K=63 bf16
    matmul == fp32-grade precision at 1 cycle/row on the PE.
  * exp(scale*psum + bias_i) with per-partition bias and fused row-sum
    (accum_out) runs entirely on the scalar engine.
  * Triangular boundary handled by a second accumulating matmul that adds +10
    to excluded psum entries -> exp arg shifted by -1000 -> exactly 0.
  * 64 row-tiles of 128 rows fall in 8 "prefix-length classes" (class s needs
    s windows of 1024 cols). Each core takes one row-tile per class -> the
    SPMD program is identical across cores; only DRAM inputs differ.
"""

import sys
import types

import numpy as np
import ml_dtypes


def _ensure_axon_hooks():
    """bass_utils' axon trace path imports antenv.axon_hooks; some images
    lack it. Install a minimal shim (hook=None -> tracing skipped)."""
    try:
        import antenv.axon_hooks  # noqa: F401
        return
    except ImportError:
        pass
    try:
        import antenv
    except ImportError:
        return
    mod = types.ModuleType("antenv.axon_hooks")
    mod._hook = None

    def set_axon_ntff_profile_hook(h):
        mod._hook = h

    def get_axon_ntff_profile_hook():
        return mod._hook

    mod.set_axon_ntff_profile_hook = set_axon_ntff_profile_hook
    mod.get_axon_ntff_profile_hook = get_axon_ntff_profile_hook
    sys.modules["antenv.axon_hooks"] = mod
    antenv.axon_hooks = mod


_ensure_axon_hooks()

import concourse.bass as bass
import concourse.bacc as bacc
import concourse.tile as tile
import concourse.mybir as mybir
import concourse.bass_utils as bass_utils

N = 8192
NCORES = 8
P = 128           # partitions / rows per tile
SLOTS = 8         # row-tiles per core (one per class)
W = 1024          # window width (cols)
K3 = 63           # 9 limb-pair groups x 7
DELTA = 0.1
SIGMA = 0.05
ACT_SCALE = -1.0 / (DELTA * DELTA)   # -100.0
FINAL_SCALE = -1.0 / SIGMA           # -20.0
MASK_ADD = 10.0   # psum bump for excluded entries: arg += ACT_SCALE*10 = -1000

BF16 = mybir.dt.bfloat16
F32 = mybir.dt.float32
NPBF16 = ml_dtypes.bfloat16

_cache = {}


def _act_units():
    """List of (slot s in 1..8, first window w0, n windows, partial col)."""
    units = []
    pc = 0
    for s in range(1, SLOTS + 1):
        w = 0
        while w < s:
            nw = min(2, s - w)
            units.append((s, w, nw, pc))
            pc += 1
            w += nw
    units.sort(key=lambda t: (t[1], -t[0]))
    return units, pc


def _build_body(tc, d):
    nc = tc.nc
    units, npart = _act_units()

    with tc.tile_pool(name="const", bufs=1) as cpool, \
         tc.tile_pool(name="work", bufs=2) as wpool, \
         tc.tile_pool(name="psum", bufs=2, space="PSUM") as ppool:

        # lhsT/rhs carry the 63 contraction rows duplicated at partition 0 and
        # 64 so consecutive matmuls can alternate PE row-groups (LDWEIGHTS of
        # one group overlaps the other group's streaming; the two groups run
        # concurrently on distinct 32x32 sub-arrays).
        # "bundle" packs lhsT | maskeye | umask into one bf16 DMA;
        # "fbundle" packs bias | srow into one f32 DMA.
        bundle_sb = cpool.tile([P, SLOTS * P + P + W], BF16, tag="bundle")
        lhsT_sb = bundle_sb[:, 0: SLOTS * P]
        mi_sb = bundle_sb[:, SLOTS * P: SLOTS * P + P]
        u_sb = bundle_sb[:, SLOTS * P + P: SLOTS * P + P + W]
        rhs_sb = cpool.tile([P, N], BF16, tag="rhs")
        fbundle_sb = cpool.tile([P, 2 * SLOTS], F32, tag="fbundle")
        bias_sb = fbundle_sb[:, 0:SLOTS]
        srow_sb = fbundle_sb[:, SLOTS: 2 * SLOTS]
        partials = cpool.tile([P, npart], F32, tag="partials")
        penalty = cpool.tile([P, SLOTS], F32, tag="penalty")
        decay = cpool.tile([P, SLOTS], F32, tag="decay")
        out_sb = cpool.tile([P, SLOTS], F32, tag="outsb")

        # first 512 cols of rhs land first so matmul 0 starts ASAP
        nc.sync.dma_start(rhs_sb[:, 0:512], d["rhs"].ap()[:, 0:512])
        nc.sync.dma_start(bundle_sb[:], d["bundle"].ap())
        nc.sync.dma_start(rhs_sb[:, 512:2048], d["rhs"].ap()[:, 512:2048])
        nc.sync.dma_start(fbundle_sb[:], d["fbundle"].ap())
        for w in range(1, 4):
            nc.sync.dma_start(
                rhs_sb[:, bass.ts(w, 2 * W)], d["rhs"].ap()[:, bass.ts(w, 2 * W)]
            )

        mm_count = 0
        for (s, w0, nw, pc) in units:
            pt = ppool.tile([P, nw * W], F32, tag="pt")
            for k in range(nw):
                w = w0 + k
                masked = (w == s - 1)
                for h in range(2):
                    base = 64 * (mm_count % 2)
                    mm_count += 1
                    psl = pt[:, k * W + 512 * h: k * W + 512 * h + 512]
                    nc.tensor.matmul(
                        psl,
                        lhsT=lhsT_sb[base: base + K3, bass.ts(s - 1, P)],
                        rhs=rhs_sb[base: base + K3, w * W + 512 * h: w * W + 512 * h + 512],
                        start=True,
                        stop=not masked,
                        tile_position=(base, 0),
                    )
                    if masked:
                        nc.tensor.matmul(
                            psl,
                            lhsT=mi_sb[:],
                            rhs=u_sb[:, 512 * h: 512 * h + 512],
                            start=False,
                            stop=True,
                        )
            e_t = wpool.tile([P, nw * W], F32, tag="e")
            nc.scalar.activation(
                e_t[:],
                pt[:],
                mybir.ActivationFunctionType.Exp,
                bias=bias_sb[:, s - 1: s],
                scale=ACT_SCALE,
                accum_out=partials[:, pc: pc + 1],
            )

        # per-slot partial columns are contiguous: offsets by slot
        off = 0
        for s in range(1, SLOTS + 1):
            naus = (s + 1) // 2
            nc.vector.tensor_reduce(
                penalty[:, s - 1: s],
                partials[:, off: off + naus],
                axis=mybir.AxisListType.X,
                op=mybir.AluOpType.add,
            )
            off += naus

        nc.scalar.activation(
            decay[:], penalty[:], mybir.ActivationFunctionType.Exp,
            scale=FINAL_SCALE,
        )
        nc.vector.tensor_mul(out_sb[:], decay[:], srow_sb[:])

        # single DMA: out[s*128 + p] = out_sb[p, s]; iterate p-major on both
        out_ap = d["out"].ap().rearrange("(s p) -> p s", p=P)
        nc.sync.dma_start(out_ap, out_sb[:])


def _build():
    if "nc" in _cache:
        return _cache["nc"]
    nc = bacc.Bacc(
        "TRN2",
        target_bir_lowering=False,
        debug=False,
        enable_asserts=False,
    )
    d = {
        "bundle": nc.dram_tensor(
            "bundle", [P, SLOTS * P + P + W], BF16, kind="ExternalInput"
        ),
        "rhs": nc.dram_tensor("rhs", [P, N], BF16, kind="ExternalInput"),
        "fbundle": nc.dram_tensor("fbundle", [P, 2 * SLOTS], F32, kind="ExternalInput"),
        "out": nc.dram_tensor("out", [SLOTS * P], F32, kind="ExternalOutput"),
    }
    with tile.TileContext(nc) as tc:
        _build_body(tc, d)
    nc.compile()
    _cache["nc"] = nc
    return nc


def _split3(x64):
    """fp64 array -> three bf16 limbs summing to ~24-bit accuracy."""
    a0 = x64.astype(NPBF16)
    r = x64 - a0.astype(np.float64)
    a1 = r.astype(NPBF16)
    r2 = r - a1.astype(np.float64)
    a2 = r2.astype(NPBF16)
    return a0, a1, a2


def _prepare_inputs(src_points, tgt_points, scores):
    scores = np.asarray(scores, np.float32)
    src = np.asarray(src_points, np.float32)
    tgt = np.asarray(tgt_points, np.float32)

    order = np.argsort(-scores.astype(np.float64), kind="stable")
    s_sorted = scores[order]
    P6 = np.concatenate([src, tgt], axis=1).astype(np.float64)[order]  # [N,6]
    sq = np.sum(P6 * P6, axis=1)  # [N] fp64

    B7 = np.concatenate([P6.T, sq[None, :]], axis=0)               # [7,N]
    A7 = np.concatenate([(-2.0 * P6).T, np.ones((1, N))], axis=0)  # [7,N]
    A0, A1, A2 = _split3(A7)
    B0, B1, B2 = _split3(B7)
    # 9 limb-pair products: (A_p)·(B_q) for p,q in 0..2
    lhsT_full = np.concatenate([A0, A0, A0, A1, A1, A1, A2, A2, A2], axis=0)  # [63,N]
    rhs63 = np.concatenate([B0, B1, B2, B0, B1, B2, B0, B1, B2], axis=0)  # [63,N]

    def dup_halves(m63):
        """[63,N] -> [128,N] with copies at partition 0 and 64 (row-group alt)."""
        out = np.zeros((P, m63.shape[1]), m63.dtype)
        out[0:K3] = m63
        out[64:64 + K3] = m63
        return np.ascontiguousarray(out)

    rhs128 = dup_halves(rhs63)

    bias_full = (ACT_SCALE * sq).astype(np.float32)  # -100*sq_i

    mi = np.ascontiguousarray((MASK_ADD * np.eye(P)).astype(NPBF16))

    in_maps = []
    for c in range(NCORES):
        gs = 8 * np.arange(SLOTS) + c  # global row-tile per slot
        rows = (gs[:, None] * P + np.arange(P)[None, :]).reshape(-1)  # [1024]
        lhsT_c = dup_halves(lhsT_full[:, rows])  # [128, 1024]
        bias_c = bias_full[rows].reshape(SLOTS, P).T  # [128,8]
        srow_c = s_sorted[rows].reshape(SLOTS, P).T.astype(np.float32)
        # exclude j >= i within masked window: local col f >= 128*c + p
        f = np.arange(W)[None, :]
        p = np.arange(P)[:, None]
        u_c = (f >= (P * c + p)).astype(NPBF16)
        bundle_c = np.ascontiguousarray(np.concatenate([lhsT_c, mi, u_c], axis=1))
        fbundle_c = np.ascontiguousarray(np.concatenate([bias_c, srow_c], axis=1))
        in_maps.append({
            "bundle": bundle_c,
            "rhs": rhs128,
            "fbundle": fbundle_c,
        })
    return in_maps, order, s_sorted, P6


def _tie_correction(out_sorted, s_sorted, P6):
    """Device counts all j<i; truth excludes tied j. Multiply by exp(+corr/SIGMA)."""
    ties = np.flatnonzero(np.diff(s_sorted) == 0.0)
    if ties.size == 0:
        return out_sorted
    # group runs of equal scores
    out = out_sorted.copy()
    runs = []
    start = ties[0]
    prev = ties[0]
    for t in ties[1:]:
        if t != prev + 1:
            runs.append((start, prev + 1))
            start = t
        prev = t
    runs.append((start, prev + 1))
    for (a, b) in runs:  # indices a..b inclusive tie group
        idx = np.arange(a, b + 1)
        for ii in range(1, idx.size):
            i = idx[ii]
            js = idx[:ii]
            d2 = np.sum((P6[i] - P6[js]) ** 2, axis=1)  # src+tgt joint
            corr = np.sum(np.exp(d2 * ACT_SCALE))
            out[i] = out[i] * np.exp(-FINAL_SCALE * corr)
    return out


LAST_EXEC_TIME_NS = None


def kernel(src_points, tgt_points, scores):
    global LAST_EXEC_TIME_NS
    nc = _build()
    in_maps, order, s_sorted, P6 = _prepare_inputs(src_points, tgt_points, scores)
    res = bass_utils.run_bass_kernel_spmd(nc, in_maps, core_ids=list(range(NCORES)))
    LAST_EXEC_TIME_NS = res.exec_time_ns

    out_sorted = np.empty((N // P, P), np.float32)
    for c in range(NCORES):
        gs = 8 * np.arange(SLOTS) + c
        out_sorted[gs, :] = np.asarray(res.results[c]["out"]).reshape(SLOTS, P)
    out_sorted = out_sorted.reshape(N)
    out_sorted = _tie_correction(out_sorted, s_sorted, P6)

    out = np.empty(N, np.float32)
    out[order] = out_sorted
    return out



# revision 10
# speedup vs baseline: 1.2964x; 1.2964x over previous
"""Correspondence Soft-NMS on 8 Trainium2 NeuronCores (Bass/Tile).

Math: penalty_i = sum_j [s_j > s_i] * exp(-(d2src_ij + d2tgt_ij)/delta^2)
      out_i    = s_i * exp(-penalty_i / sigma)

Strategy (v2):
  * Host sorts by score desc; suppressors of row i are the strict prefix
    {j < i} (exact ties fixed by a host-side correction, as before).
  * K=42 bf16 matmul: 6 limb-pair groups for -2*x_i.x_j (6 dims), 3 rows
    for sq_j limbs, 3 rows for (sq_i + cK) limbs.  psum = d2_ij + cK where
    cK = B16/A16 folds the Schraudolph offset into the matmul so every
    elementwise consumer needs only immediate scalars.
  * Per-core poison-shift: core c's rhs is the sorted column stream shifted
    right by o_c = 896-128c with poison columns (huge sq) in front.  Then
    slot k (row-tile 8k+c) sums exactly windows [0, 1024(k+1)) for EVERY
    core, the triangular boundary always lands in the last 128 columns of
    the last window (one F=128 eye-bump matmul), and no other masking or
    per-core shapes are needed.
  * exp+row-sum is split across THREE engines to beat the scalar-engine
    ceiling: ACT does exp with fused accum (exact); DVE and Pool compute
    i16 = max(psum*A16, 0) whose bit pattern IS bf16 exp (Schraudolph),
    then DVE row-sums those via a 4x-mode bypass tensor_scalar with
    accum_out.  Clamp-at-0 makes out-of-range / poisoned / masked entries
    contribute exactly +0.0.
  * psum is a 4-deep ring of [128,1024] tiles so the PE streams ahead of
    the three consumers.
"""

import sys
import types

import numpy as np
import ml_dtypes


def _ensure_axon_hooks():
    """bass_utils' axon trace path imports antenv.axon_hooks; some images
    lack it. Install a minimal shim (hook=None -> tracing skipped)."""
    try:
        import antenv.axon_hooks  # noqa: F401
        return
    except ImportError:
        pass
    try:
        import antenv
    except ImportError:
        return
    mod = types.ModuleType("antenv.axon_hooks")
    mod._hook = None

    def set_axon_ntff_profile_hook(h):
        mod._hook = h

    def get_axon_ntff_profile_hook():
        return mod._hook

    mod.set_axon_ntff_profile_hook = set_axon_ntff_profile_hook
    mod.get_axon_ntff_profile_hook = get_axon_ntff_profile_hook
    sys.modules["antenv.axon_hooks"] = mod
    antenv.axon_hooks = mod


_ensure_axon_hooks()

import concourse.bass as bass
import concourse.bacc as bacc
import concourse.tile as tile
import concourse.mybir as mybir
import concourse.bass_utils as bass_utils

N = 8192
NCORES = 8
P = 128
SLOTS = 8
W = 1024
K1 = 42           # contraction rows (single copy)
DELTA = 0.1
SIGMA = 0.05
ACT_SCALE = -1.0 / (DELTA * DELTA)   # -100.0
FINAL_SCALE = -1.0 / SIGMA           # -20.0

LN2 = float(np.log(2.0))
A16 = ACT_SCALE * 128.0 / LN2        # psum -> i16 scale (-18466.27)
CTUNE = 6.83                          # Schraudolph bias tuning (floor conv)
B16 = 16256.0 - CTUNE
CK = B16 / A16                        # folded into sq_i rows (negative)
ACT_BIAS = -ACT_SCALE * CK           # exp(ACT_SCALE*psum + ACT_BIAS) == exp(ACT_SCALE*d2)
POISON = 3000.0

BF16 = mybir.dt.bfloat16
F32 = mybir.dt.float32
I16 = mybir.dt.int16
NPBF16 = ml_dtypes.bfloat16

# per-(slot,window) consumer: 'A' scalar/ACT exp, 'D' vector/DVE exp.
# (GPSIMD/Pool can neither read PSUM nor free-axis-reduce on TRN2, so DVE
# row-sums its own bf16 exp output via a 4x-mode bypass tensor_scalar.)
ASSIGN = [
    "D",
    "AD",
    "ADA",
    "ADAA",
    "ADADA",
    "ADADAA",
    "ADADADA",
    "ADADADAD",
]

_cache = {}


def _build_body(tc, d):
    nc = tc.nc

    with tc.tile_pool(name="const", bufs=1) as cpool, \
         tc.tile_pool(name="ascr", bufs=2) as apool, \
         tc.tile_pool(name="dscr", bufs=2) as dpool, \
         tc.tile_pool(name="rscr", bufs=2) as rpool, \
         tc.tile_pool(name="psum", bufs=4, space="PSUM") as pspool:

        # wbundle: mi(128) | u(128) | lhsT slots (8*128)
        wbundle = cpool.tile([P, 2 * P + SLOTS * P], BF16, tag="wbundle")
        mi_sb = wbundle[:, 0:P]
        u_sb = wbundle[:, P: 2 * P]
        lhsT_sb = wbundle[:, 2 * P: 2 * P + SLOTS * P]
        rhs_sb = cpool.tile([P, N], BF16, tag="rhs")
        srow_sb = cpool.tile([P, SLOTS], F32, tag="srow")
        partials = cpool.tile([P, SLOTS * SLOTS], F32, tag="partials")
        biast = cpool.tile([P, 1], F32, tag="biast")
        penalty = cpool.tile([P, SLOTS], F32, tag="penalty")
        decay = cpool.tile([P, SLOTS], F32, tag="decay")
        out_sb = cpool.tile([P, SLOTS], F32, tag="outsb")

        nc.gpsimd.memset(partials[:], 0.0)
        nc.gpsimd.memset(biast[:], float(ACT_BIAS))

        # hot first: mi|u + first 3 slot weights, then first rhs window
        nc.sync.dma_start(wbundle[:, 0: 2 * P + 3 * P], d["wbundle"].ap()[:, 0: 2 * P + 3 * P])
        nc.sync.dma_start(rhs_sb[:, 0:2048], d["rhs"].ap()[:, 0:2048])
        nc.sync.dma_start(
            wbundle[:, 2 * P + 3 * P:], d["wbundle"].ap()[:, 2 * P + 3 * P:]
        )
        nc.sync.dma_start(srow_sb[:], d["srow"].ap())
        nc.sync.dma_start(rhs_sb[:, 2048:4096], d["rhs"].ap()[:, 2048:4096])
        nc.sync.dma_start(rhs_sb[:, 4096:N], d["rhs"].ap()[:, 4096:N])

        mm = 0
        for k in range(SLOTS):
            for w in range(k + 1):
                pt = pspool.tile([P, W], F32, tag="pt")
                masked = (w == k)
                # column spans within the window; the tri-boundary span
                # [896:1024] of a masked window stays open for the eye-bump
                spans = [(0, 512), (512, 1024)]
                if masked:
                    spans = [(0, 512), (512, 896), (896, 1024)]
                for (lo, hi) in spans:
                    base = 64 * (mm % 2)
                    mm += 1
                    nc.tensor.matmul(
                        pt[:, lo:hi],
                        lhsT=lhsT_sb[base: base + K1, bass.ts(k, P)],
                        rhs=rhs_sb[base: base + K1, w * W + lo: w * W + hi],
                        start=True,
                        stop=not (masked and hi == 1024),
                        tile_position=(base, 0),
                    )
                if masked:
                    nc.tensor.matmul(
                        pt[:, 896:1024],
                        lhsT=mi_sb[:],
                        rhs=u_sb[:],
                        start=False,
                        stop=True,
                    )
                eng = ASSIGN[k][w]
                pcol = partials[:, SLOTS * k + w: SLOTS * k + w + 1]
                if eng == "A":
                    ea = apool.tile([P, W], BF16, tag="ea")
                    nc.scalar.activation(
                        ea[:], pt[:], mybir.ActivationFunctionType.Exp,
                        bias=biast[:], scale=ACT_SCALE, accum_out=pcol,
                    )
                else:
                    ei = dpool.tile([P, W], I16, tag="ei")
                    nc.vector.tensor_scalar(
                        ei[:], pt[:], float(A16), 0.0,
                        op0=mybir.AluOpType.mult, op1=mybir.AluOpType.max,
                    )
                    rs = rpool.tile([P, W], BF16, tag="rs")
                    nc.vector.tensor_scalar(
                        rs[:], ei[:].bitcast(BF16), 1.0, None,
                        op0=mybir.AluOpType.mult, op1=mybir.AluOpType.add,
                        accum_out=pcol,
                    )

        pr = partials[:].rearrange("p (s w) -> p s w", w=SLOTS)
        nc.vector.tensor_reduce(
            penalty[:], pr, axis=mybir.AxisListType.X, op=mybir.AluOpType.add
        )
        nc.scalar.activation(
            decay[:], penalty[:], mybir.ActivationFunctionType.Exp,
            scale=FINAL_SCALE,
        )
        nc.vector.tensor_mul(out_sb[:], decay[:], srow_sb[:])

        out_ap = d["out"].ap().rearrange("(p s) -> p s", s=SLOTS)
        nc.sync.dma_start(out_ap, out_sb[:])


def _build():
    if "nc" in _cache:
        return _cache["nc"]
    nc = bacc.Bacc(
        "TRN2",
        target_bir_lowering=False,
        debug=False,
        enable_asserts=False,
    )
    d = {
        "wbundle": nc.dram_tensor(
            "wbundle", [P, 2 * P + SLOTS * P], BF16, kind="ExternalInput"
        ),
        "rhs": nc.dram_tensor("rhs", [P, N], BF16, kind="ExternalInput"),
        "srow": nc.dram_tensor("srow", [P, SLOTS], F32, kind="ExternalInput"),
        "out": nc.dram_tensor("out", [P * SLOTS], F32, kind="ExternalOutput"),
    }
    with tile.TileContext(nc) as tc:
        _build_body(tc, d)
    nc.compile()
    _cache["nc"] = nc
    return nc


def _split3(x64):
    """fp64 array -> three bf16 limbs summing to ~24-bit accuracy."""
    a0 = x64.astype(NPBF16)
    r = x64 - a0.astype(np.float64)
    a1 = r.astype(NPBF16)
    r2 = r - a1.astype(np.float64)
    a2 = r2.astype(NPBF16)
    return a0, a1, a2


# limb-pair groups for the -2x.y part
PQ = [(0, 0), (0, 1), (1, 0), (1, 1), (0, 2), (2, 0)]


def _prepare_inputs(src_points, tgt_points, scores):
    scores = np.asarray(scores, np.float32)
    src = np.asarray(src_points, np.float32)
    tgt = np.asarray(tgt_points, np.float32)

    order = np.argsort(-scores.astype(np.float64), kind="stable")
    s_sorted = scores[order]
    P6 = np.concatenate([src, tgt], axis=1).astype(np.float64)[order]  # [N,6]
    sq = np.sum(P6 * P6, axis=1)  # [N] fp64

    A_l = _split3((-2.0 * P6).T)   # 3 x [6,N]
    B_l = _split3(P6.T)            # 3 x [6,N]
    sqj_l = _split3(sq[None, :])   # 3 x [1,N]
    sqi_l = _split3((sq + CK)[None, :])  # 3 x [1,N]

    ones = np.ones((1, N), NPBF16)
    zeros = np.zeros((1, N), NPBF16)

    # A-side rows [42, N]
    A_rows = np.concatenate(
        [A_l[p] for (p, q) in PQ] + [ones, ones, ones] + [sqi_l[0], sqi_l[1], sqi_l[2]],
        axis=0,
    )
    # B-side rows [42, N]
    B_rows = np.concatenate(
        [B_l[q] for (p, q) in PQ] + [sqj_l[0], sqj_l[1], sqj_l[2]] + [ones, ones, ones],
        axis=0,
    )

    def dup(m):
        out = np.zeros((P, m.shape[1]), NPBF16)
        out[0:K1] = m
        out[64:64 + K1] = m
        return out

    mi = (10.0 * np.eye(P)).astype(NPBF16)
    f = np.arange(P)[None, :]
    p_ = np.arange(P)[:, None]
    u = (f >= p_).astype(NPBF16)

    # poison column (contributes exp(-1e5)==0 on every consumer path)
    poison = np.zeros((K1, 1), NPBF16)
    poison[len(PQ) * 6] = POISON  # sq_j limb0 row; pairs with A ones row

    in_maps = []
    for c in range(NCORES):
        oc = 896 - 128 * c
        rhs_c = np.zeros((K1, N), NPBF16)
        rhs_c[:, 0:oc] = poison
        rhs_c[:, oc:N] = B_rows[:, 0:N - oc]
        rows = (
            (8 * np.arange(SLOTS)[:, None] + c) * P + np.arange(P)[None, :]
        ).reshape(-1)  # [1024] sorted-row indices, slot-major
        lhsT_c = A_rows[:, rows]  # [42, 1024]
        # mi/u occupy all 128 partitions; weights dup'd at partition 0 and 64
        wb = np.zeros((P, 2 * P + SLOTS * P), NPBF16)
        wb[:, 0:P] = mi
        wb[:, P:2 * P] = u
        wb[0:K1, 2 * P:] = lhsT_c
        wb[64:64 + K1, 2 * P:] = lhsT_c
        srow_c = s_sorted[rows].reshape(SLOTS, P).T.astype(np.float32)
        in_maps.append({
            "wbundle": np.ascontiguousarray(wb),
            "rhs": np.ascontiguousarray(dup(rhs_c)),
            "srow": np.ascontiguousarray(srow_c),
        })
    return in_maps, order, s_sorted, P6


def _tie_correction(out_sorted, s_sorted, P6):
    """Device counts all j<i; truth excludes tied j. Multiply by exp(+corr/SIGMA)."""
    ties = np.flatnonzero(np.diff(s_sorted) == 0.0)
    if ties.size == 0:
        return out_sorted
    out = out_sorted.copy()
    runs = []
    start = ties[0]
    prev = ties[0]
    for t in ties[1:]:
        if t != prev + 1:
            runs.append((start, prev + 1))
            start = t
        prev = t
    runs.append((start, prev + 1))
    for (a, b) in runs:  # indices a..b inclusive tie group
        idx = np.arange(a, b + 1)
        for ii in range(1, idx.size):
            i = idx[ii]
            js = idx[:ii]
            d2 = np.sum((P6[i] - P6[js]) ** 2, axis=1)
            corr = np.sum(np.exp(d2 * ACT_SCALE))
            out[i] = out[i] * np.exp(-FINAL_SCALE * corr)
    return out


def _assemble(core_outs, order, s_sorted, P6):
    """core_outs[c]: flat [P*SLOTS] device output laid out (p, s)."""
    out_sorted = np.empty(N, np.float32)
    for c in range(NCORES):
        oc = np.asarray(core_outs[c], np.float32).reshape(P, SLOTS)
        rows = (8 * np.arange(SLOTS)[None, :] + c) * P + np.arange(P)[:, None]
        out_sorted[rows.reshape(-1)] = oc.reshape(-1)
    out_sorted = _tie_correction(out_sorted, s_sorted, P6)
    out = np.empty(N, np.float32)
    out[order] = out_sorted
    return out


LAST_EXEC_TIME_NS = None


def kernel(src_points, tgt_points, scores):
    global LAST_EXEC_TIME_NS
    nc = _build()
    in_maps, order, s_sorted, P6 = _prepare_inputs(src_points, tgt_points, scores)
    res = bass_utils.run_bass_kernel_spmd(nc, in_maps, core_ids=list(range(NCORES)))
    LAST_EXEC_TIME_NS = res.exec_time_ns
    return _assemble(
        [res.results[c]["out"] for c in range(NCORES)], order, s_sorted, P6
    )
